# revision 37
# baseline (speedup 1.0000x reference)
"""DirectedDualSAGE (2-layer dual-direction GraphSAGE + MLP head) on 8 trn2
NeuronCores via Bass/Tile.

Sharding: nodes (dsts) block-partitioned 6250/core; each core owns all edges
whose dst lies in its shard, for both edge directions.

Per layer and direction, lin_l(mean_j x_j) is computed as
diag(1/cnt) * A * (x @ Wl): transform first (64-dim), halving gather traffic.
The segment-mean is a per-edge row gather (dma_gather, 4 SWDGE queues
round-robin) from a DRAM feature table + prefix-structured accumulation on
the vector engine: dsts are sorted by descending degree so "round r" (the
r-th edge of every dst) occupies slot prefix [0, n_r) and accumulates with
one contiguous tensor_tensor add per gather fragment. Gather indices are
int16, so edges are split by src half (< / >= 25000) with per-half tables,
accumulators and zero-pad rows; accumulators are unpermuted back to natural
dst order with a second (6272-row) dma_gather and merged.

Layer 1's tables (x @ [Wl_in|Wl_out], one per half) are built replicated on
every core (x is an input -> no communication); the A-half gathers overlap
the B-half build. Layer 2's tables are built from the local x2 shard and
AllGather'ed (one collective per direction, mesh ~40us each).

Dense math runs feature-major on the tensor engine; aggregated means
(node-major) are transposed back via PE identity-matmuls accumulating into
the same PSUM as the x @ Wr term, then bias+relu on the scalar engine.

kernel(**inputs) takes full unsharded inputs, returns the full [N] output.
"""
import numpy as np

import concourse.bacc as bacc
import concourse.tile as tile
import concourse.mybir as mybir
from concourse import bass_utils

F32 = mybir.dt.float32
I16 = mybir.dt.int16

N = 50000
NC = 8
NLOC = N // NC            # 6250
NLOCP = 6272              # 49*128
NCH = NLOCP // 128        # 49 chunks
XCOLS = 50088             # 25000 + 49*512 (xt_full padded cols)
HALF = 25000              # src half split (rank-aligned)
ZHEAD = 128               # zero rows at table head
HROWS = ZHEAD + HALF + 176     # 25304 rows per half table
TROWS = ZHEAD + N + 176        # 50304 rows for the AllGather'ed Y2 tables
BZERO = ZHEAD + HALF      # 25128: B-half zero idx (y1B tail / Y2 row 50128)
SMAX = 3200               # max rows per dma_gather call
NQ = 4                    # SWDGE queues

_CACHE = {}
DEBUG = False


# ----------------------------------------------------------------- host prep

def _round_up(v, m):
    return (v + m - 1) // m * m


def _per_core_half(src, dst, half_mask):
    out = []
    for c in range(NC):
        m = (dst // NLOC == c) & half_mask
        s = src[m]
        dloc = (dst[m] - c * NLOC).astype(np.int64)
        deg = np.bincount(dloc, minlength=NLOCP).astype(np.int64)
        perm = np.argsort(-deg, kind="stable").astype(np.int64)
        pos = np.empty(NLOCP, dtype=np.int64)
        pos[perm] = np.arange(NLOCP)
        order = np.argsort(dloc, kind="stable")
        sd = dloc[order]
        ss = s[order]
        if len(sd):
            starts = np.r_[0, 1 + np.flatnonzero(np.diff(sd))]
            group_id = np.zeros(len(sd), dtype=np.int64)
            group_id[starts[1:]] = 1
            group_id = np.cumsum(group_id)
            rank = np.arange(len(sd)) - starts[group_id]
        else:
            rank = sd
        slot = pos[sd]
        maxdeg = int(deg.max()) if len(sd) else 0
        rounds = []
        for r in range(maxdeg):
            mr = rank == r
            rounds.append((int(np.count_nonzero(mr)), slot[mr], ss[mr]))
        out.append(dict(deg=deg, pos=pos, rounds=rounds))
    return out


def _preprocess(edge_index_in, edge_index_out):
    plan = {"dirs": {}}
    for dname, ei in (("in", edge_index_in), ("out", edge_index_out)):
        src = ei[0].astype(np.int64)
        dst = ei[1].astype(np.int64)
        dinfo = {"halves": {}, "recip": []}
        for c in range(NC):
            m = dst // NLOC == c
            dloc = dst[m] - c * NLOC
            cnt = np.bincount(dloc, minlength=NLOCP).astype(np.float32)
            dinfo["recip"].append((1.0 / np.maximum(cnt, 1.0)).astype(np.float32))
        for hname, is_a in (("A", True), ("B", False)):
            half_mask = (src < HALF) if is_a else (src >= HALF)
            cores = _per_core_half(src, dst, half_mask)
            nrounds = max(len(ci["rounds"]) for ci in cores)
            NR = []
            for r in range(nrounds):
                mx = max((ci["rounds"][r][0] if r < len(ci["rounds"]) else 0)
                         for ci in cores)
                NR.append(_round_up(max(mx, 1), 128))
            NR[0] = NLOCP  # full first round: copy-initializes the accumulator
            zi = 0 if is_a else BZERO
            streams = []
            for ci in cores:
                parts = []
                for r in range(nrounds):
                    vec = np.full(NR[r], zi, dtype=np.int64)
                    if r < len(ci["rounds"]):
                        _, slots, ss = ci["rounds"][r]
                        vec[slots] = (ss + ZHEAD) if is_a else (ss - HALF + ZHEAD)
                    parts.append(vec)
                stream = np.concatenate(parts)
                assert stream.max(initial=0) < 32768
                streams.append(stream.astype(np.int16))
            # uniform SMAX-row cuts; rounds may split across groups (the
            # per-fragment add targets acc slot range [s0, s1))
            L = int(sum(NR))
            groups = []  # (stream_off, [(stg_off, acc_slot_off, nrows, r)])
            r, r_off = 0, 0
            off = 0
            while off < L:
                rows = min(SMAX, L - off)
                frags = []
                done = 0
                while done < rows:
                    take = min(NR[r] - r_off, rows - done)
                    frags.append((done, r_off, take, r))
                    done += take
                    r_off += take
                    if r_off == NR[r]:
                        r += 1
                        r_off = 0
                groups.append((off, frags))
                off += rows
            dinfo["halves"][hname] = dict(
                NR=NR, L=int(sum(NR)), streams=streams, groups=groups,
                unperm=[ci["pos"].astype(np.int16) for ci in cores], is_a=is_a,
            )
        plan["dirs"][dname] = dinfo
    return plan


def _wrap_idx(idx):
    L = idx.shape[0]
    assert L % 16 == 0
    w = idx.reshape(L // 16, 16).T.astype(np.int16)
    return np.ascontiguousarray(np.tile(w, (8, 1)))


# ------------------------------------------------------------- device program

def _build_program(plan):
    nc = bacc.Bacc("TRN2", target_bir_lowering=False, debug=False,
                   num_devices=NC, num_swdge_queues=NQ)
    dims = ("in", "out")
    inp = {}

    def dram_in(name, shape, dt=F32):
        inp[name] = nc.dram_tensor(name, list(shape), dt, kind="ExternalInput")
        return inp[name]

    xt_full = dram_in("xt_full", [128, XCOLS])
    xt_loc = dram_in("xt_loc", [128, NLOCP])
    ident = dram_in("ident", [128, 128])
    for li in (1, 2):
        dram_in(f"wl_comb{li}", [128, 128])
        dram_in(f"wr_in{li}", [128, 64])
        dram_in(f"wr_out{li}", [128, 64])
        dram_in(f"bias_pk{li}", [128, 1])
        dram_in(f"wcx{li}", [128, 128])
        dram_in(f"wch{li}", [128, 128])
        dram_in(f"cb{li}", [128, 1])
    dram_in("fw", [128, 1])
    dram_in("fb", [1, 1])
    for d in dims:
        dram_in(f"recip_{d}", [128, NCH])
        for h in ("A", "B"):
            L = plan["dirs"][d]["halves"][h]["L"]
            dram_in(f"stream_{d}_{h}", [128, L // 16], I16)
            dram_in(f"unperm_{d}_{h}", [128, NLOCP // 16], I16)

    y1t = {(h, d): nc.dram_tensor(f"y1{h}{d}", [HROWS, 64], F32,
                                  kind="Internal")
           for h in ("A", "B") for d in dims}
    y2t = {d: nc.dram_tensor(f"y2{d}", [TROWS, 64], F32, kind="Internal",
                             addr_space="Shared") for d in dims}
    y2sl = {d: nc.dram_tensor(f"y2sl{d}", [NLOCP, 64], F32, kind="Internal")
            for d in dims}
    accd = {d: nc.dram_tensor(f"accd_{d}", [2, NLOCP, 64], F32, kind="Internal")
            for d in dims}
    out_t = nc.dram_tensor("out", [1, NLOC], F32, kind="ExternalOutput")

    _qctr = [0]

    def next_queue():
        q = _qctr[0] % NQ
        _qctr[0] += 1
        return q

    RELU = mybir.ActivationFunctionType.Relu
    COPY = mybir.ActivationFunctionType.Copy

    with tile.TileContext(nc) as tc:
        with tc.tile_pool(name="const", bufs=1) as cpool, \
             tc.tile_pool(name="idxp", bufs=1) as idxp, \
             tc.tile_pool(name="idxg", bufs=8) as idxgp, \
             tc.tile_pool(name="feat", bufs=2) as featp, \
             tc.tile_pool(name="accp", bufs=2) as accp, \
             tc.tile_pool(name="meanp", bufs=1) as meanp, \
             tc.tile_pool(name="stg", bufs=6) as stgp, \
             tc.tile_pool(name="natp", bufs=2) as natp, \
             tc.tile_pool(name="ybld", bufs=2) as ybldp, \
             tc.tile_pool(name="ystg", bufs=2) as ystgp, \
             tc.tile_pool(name="small", bufs=1) as smallp, \
             tc.tile_pool(name="ps", bufs=3, space="PSUM") as psp, \
             tc.tile_pool(name="psf", bufs=1, space="PSUM") as psfp, \
             tc.tile_pool(name="psy", bufs=2, space="PSUM") as psyp:

            def load_const(name, shape, dt=F32):
                t = cpool.tile(list(shape), dt, tag=name, name=f"c_{name}")
                nc.sync.dma_start(t[:], inp[name][tuple(slice(None) for _ in shape)])
                return t

            ident_t = load_const("ident", [128, 128])
            W = {}
            for li in (1, 2):
                for nm, shp in (("wl_comb", [128, 128]), ("wr_in", [128, 64]),
                                ("wr_out", [128, 64]), ("bias_pk", [128, 1]),
                                ("wcx", [128, 128]), ("wch", [128, 128]),
                                ("cb", [128, 1])):
                    W[f"{nm}{li}"] = load_const(f"{nm}{li}", shp)
            fw_t = load_const("fw", [128, 1])
            fb_t = load_const("fb", [1, 1])
            recip_t = {d: load_const(f"recip_{d}", [128, NCH]) for d in dims}
            unperm_t = {}
            for d in dims:
                for h in ("A", "B"):
                    ut = idxp.tile([128, NLOCP // 16], I16, tag=f"up_{d}_{h}", name=f"up_{d}_{h}")
                    nc.sync.dma_start(ut[:], inp[f"unperm_{d}_{h}"][:, :])
                    unperm_t[d, h] = ut

            zero_t = smallp.tile([128, 128], F32, tag="zero")
            nc.vector.memset(zero_t[:], 0.0)

            def zero_rows(tab, start, width=128):
                nc.sync.dma_start(
                    tab[start:start + 128, 0:width]
                    .rearrange("(k p) c -> p k c", p=128),
                    zero_t[:, 0:width].rearrange("p (k c) -> p k c", k=1))

            for h in ("A", "B"):
                for d in dims:
                    zero_rows(y1t[h, d], 0, 64)
                    zero_rows(y1t[h, d], HROWS - 176, 64)
                    zero_rows(y1t[h, d], HROWS - 128, 64)
            for d in dims:
                zero_rows(y2t[d], 0, 64)
                zero_rows(y2t[d], ZHEAD + N, 64)
                zero_rows(y2t[d], TROWS - 128, 64)

            # ---------------- y1 table build (replicated; per src half so
            # A-half gathers can start while the B half still builds)
            def build_half(h, col0):
                t_off = 0
                while t_off < 49 * 512:
                    tw = min(1024, 49 * 512 - t_off)
                    nchk = tw // 128
                    xs = ybldp.tile([128, 1024], F32, tag="xs")
                    nc.scalar.dma_start(
                        xs[:, 0:tw], xt_full[:, col0 + t_off:col0 + t_off + tw])
                    ps = psyp.tile([128, 1024], F32, tag="psy")
                    for k in range(nchk):
                        nc.tensor.matmul(ps[:, 128 * k:128 * (k + 1)],
                                         xs[:, 128 * k:128 * (k + 1)],
                                         W["wl_comb1"][:], start=True, stop=True)
                    ys = ystgp.tile([128, 1024], F32, tag="ys")
                    nc.scalar.activation(ys[:, 0:tw], ps[:, 0:tw], COPY)
                    ysv = ys[:, 0:tw].rearrange("p (k c) -> p k c", k=nchk)
                    for di, d in enumerate(dims):
                        nc.sync.dma_start(
                            y1t[h, d][ZHEAD + t_off:ZHEAD + t_off + tw, :]
                            .rearrange("(k p) c -> p k c", p=128),
                            ysv[:, :, 64 * di:64 * (di + 1)])
                    t_off += tw


            # ---------------- shared helpers
            hidx = {"A": 0, "B": 1}

            def gather_half(li, d, h, tab_ap, estep):
                """Gather+accumulate one (direction, half); returns acc tile."""
                hinfo = plan["dirs"][d]["halves"][h]
                acc = accp.tile([128, NCH, 64], F32, tag="acc",
                                name=f"acc{li}_{d}_{h}")
                for goff, frags in hinfo["groups"]:
                    rows = sum(f[2] for f in frags)
                    gidx = idxgp.tile([128, SMAX // 16], I16, tag="gidx")
                    nc.scalar.dma_start(
                        gidx[:, 0:rows // 16],
                        inp[f"stream_{d}_{h}"][:, goff // 16:(goff + rows) // 16])
                    stg = stgp.tile([128, SMAX // 128, 64], F32, tag="stg")
                    nc.gpsimd.dma_gather(
                        stg[:, 0:rows // 128, :], tab_ap,
                        gidx[:, 0:rows // 16],
                        num_idxs=rows, num_idxs_reg=rows,
                        elem_size=64, elem_step=estep, single_packet=False,
                        queue_num=next_queue())
                    for stg_off, slot_off, nrows, r in frags:
                        cr = nrows // 128
                        c0 = slot_off // 128
                        s_ap = stg[:, stg_off // 128:stg_off // 128 + cr, :]
                        a_ap = acc[:, c0:c0 + cr, :]
                        if r == 0:
                            nc.vector.tensor_copy(a_ap, s_ap)
                        else:
                            nc.vector.tensor_add(a_ap, a_ap, s_ap)
                nc.sync.dma_start(
                    accd[d][hidx[h], :, :].rearrange("(c p) f -> p c f", p=128),
                    acc[:])

            def finish_dir(d, mean_tiles):
                nats = {}
                for h in ("A", "B"):
                    nat = natp.tile([128, NCH, 64], F32, tag="nat")
                    nc.gpsimd.dma_gather(
                        nat[:], accd[d][hidx[h], :, :], unperm_t[d, h][:],
                        num_idxs=NLOCP, num_idxs_reg=NLOCP,
                        elem_size=64, elem_step=64, single_packet=False,
                        queue_num=next_queue())
                    nats[h] = nat
                mean = mean_tiles[d]
                nc.vector.tensor_add(mean, nats["A"][:], nats["B"][:])
                rb = recip_t[d][:].unsqueeze(2).broadcast_to((128, NCH, 64))
                nc.vector.tensor_mul(mean, mean, rb)

            def agg_layer(li, tab_fn, mean_tiles):
                gather_half(li, "in", "A", *tab_fn("in", "A"))
                gather_half(li, "in", "B", *tab_fn("in", "B"))
                gather_half(li, "out", "A", *tab_fn("out", "A"))
                finish_dir("in", mean_tiles)
                gather_half(li, "out", "B", *tab_fn("out", "B"))
                finish_dir("out", mean_tiles)

            def seg_widths():
                segs = []
                off = 0
                while off < NLOCP:
                    w = min(512, NLOCP - off)
                    segs.append((off, w))
                    off += w
                return segs

            def h_pass(li, get_feat, mean_pk, h_t):
                """h_packed = relu(meanT + Wr.T @ featT + bias)."""
                for off, w in seg_widths():
                    feat_ap = get_feat(off, w)
                    ps = psp.tile([128, 512], F32, tag="ps")
                    nch = w // 128
                    nc.tensor.matmul(ps[0:64, 0:w], W[f"wr_in{li}"][:],
                                     feat_ap, start=True, stop=False)
                    nc.tensor.matmul(ps[64:128, 0:w], W[f"wr_out{li}"][:],
                                     feat_ap, start=True, stop=False,
                                     tile_position=(0, 64))
                    for k in range(nch):
                        c = (off + 128 * k) // 128
                        nc.tensor.matmul(ps[:, 128 * k:128 * (k + 1)],
                                         mean_pk[:, c, :], ident_t[:],
                                         start=False, stop=k == nch - 1)
                    nc.scalar.activation(h_t[:, off:off + w], ps[:, 0:w], RELU,
                                         bias=W[f"bias_pk{li}"][:])

            def comb_pass(li, get_feat, h_t, out_cb):
                for off, w in seg_widths():
                    ps = psp.tile([128, 512], F32, tag="ps")
                    nc.tensor.matmul(ps[:, 0:w], W[f"wcx{li}"][:],
                                     get_feat(off, w), start=True, stop=False)
                    nc.tensor.matmul(ps[:, 0:w], W[f"wch{li}"][:],
                                     h_t[:, off:off + w], start=False, stop=True)
                    out_cb(off, w, ps)

            def xt_seg(off, w):
                xs = ybldp.tile([128, 512], F32, tag="xseg")
                nc.sync.dma_start(xs[:, 0:w], xt_loc[:, off:off + w])
                return xs[:, 0:w]

            # ---------------- layer 1
            mean_pk1 = meanp.tile([128, NCH, 128], F32, tag="mean",
                                  name="mean_pk1")
            mean_tiles = {"in": mean_pk1[:, :, 0:64],
                          "out": mean_pk1[:, :, 64:128]}

            def l1_tab(d, h):
                return y1t[h, d][0:HROWS, :], 64

            build_half("A", 0)
            gather_half(1, "in", "A", *l1_tab("in", "A"))
            build_half("B", HALF)
            gather_half(1, "in", "B", *l1_tab("in", "B"))
            gather_half(1, "out", "A", *l1_tab("out", "A"))
            finish_dir("in", mean_tiles)
            gather_half(1, "out", "B", *l1_tab("out", "B"))
            finish_dir("out", mean_tiles)
            h1_t = featp.tile([128, NLOCP], F32, tag="bigfeat")
            h_pass(1, xt_seg, mean_pk1, h1_t)
            x2_t = featp.tile([128, NLOCP], F32, tag="bigfeat")

            def l1_out(off, w, ps):
                nc.scalar.activation(x2_t[:, off:off + w], ps[:, 0:w], RELU,
                                     bias=W["cb1"][:])
            comb_pass(1, xt_seg, h1_t, l1_out)

            # y2 table slices (per direction) + AllGathers
            for g in range((NCH + 3) // 4):
                c0 = 4 * g
                ncc = min(4, NCH - c0)
                ps = psyp.tile([128, 512], F32, tag="psy")
                for k in range(ncc):
                    nc.tensor.matmul(ps[:, 128 * k:128 * (k + 1)],
                                     x2_t[:, 128 * (c0 + k):128 * (c0 + k + 1)],
                                     W["wl_comb2"][:], start=True, stop=True)
                ys = ystgp.tile([128, 512], F32, tag="ys")
                nc.scalar.activation(ys[:, 0:128 * ncc], ps[:, 0:128 * ncc], COPY)
                ysv = ys[:, 0:128 * ncc].rearrange("p (k c) -> p k c", k=ncc)
                for di, d in enumerate(dims):
                    nc.sync.dma_start(
                        y2sl[d][128 * c0:128 * (c0 + ncc), :]
                        .rearrange("(k p) c -> p k c", p=128),
                        ysv[:, :, 64 * di:64 * (di + 1)])
            for d in dims:
                nc.gpsimd.collective_compute(
                    "AllGather", mybir.AluOpType.bypass,
                    replica_groups=[list(range(NC))],
                    ins=[y2sl[d][0:NLOC, :]],
                    outs=[y2t[d][ZHEAD:ZHEAD + N, :]],
                )

            # ---------------- layer 2
            mean_pk2 = meanp.tile([128, NCH, 128], F32, tag="mean",
                                  name="mean_pk2")
            mean_tiles2 = {"in": mean_pk2[:, :, 0:64],
                           "out": mean_pk2[:, :, 64:128]}

            def l2_tab(d, h):
                if h == "A":
                    return y2t[d][0:HROWS, :], 64
                return y2t[d][HALF:TROWS, :], 64
            agg_layer(2, l2_tab, mean_tiles2)
            h2_t = featp.tile([128, NLOCP], F32, tag="bigfeat")

            def x2_seg(off, w):
                return x2_t[:, off:off + w]
            h_pass(2, x2_seg, mean_pk2, h2_t)

            def l2_out(off, w, ps):
                x3 = ystgp.tile([128, 512], F32, tag="x3")
                nc.scalar.activation(x3[:, 0:w], ps[:, 0:w], RELU,
                                     bias=W["cb2"][:])
                psf = psfp.tile([1, 512], F32, tag="psf")
                nc.tensor.matmul(psf[0:1, 0:w], fw_t[:], x3[:, 0:w],
                                 start=True, stop=True)
                osb = ystgp.tile([1, 512], F32, tag="osb")
                nc.vector.tensor_scalar_add(osb[0:1, 0:w],
                                            psf[0:1, 0:w], fb_t[0:1, 0:1])
                wv = min(w, NLOC - off)
                if wv > 0:
                    nc.sync.dma_start(out_t[0:1, off:off + wv], osb[0:1, 0:wv])
            comb_pass(2, x2_seg, h2_t, l2_out)

    nc.compile()
    return nc


# ------------------------------------------------------------------ interface

def _make_in_maps(plan, inputs):
    x = np.asarray(inputs["x"], dtype=np.float32)
    xt = np.zeros((128, XCOLS), dtype=np.float32)
    xt[:, :N] = np.ascontiguousarray(x.T)
    ident = np.eye(128, dtype=np.float32)

    def cat(a, b):
        return np.ascontiguousarray(
            np.concatenate([np.asarray(a, np.float32), np.asarray(b, np.float32)],
                           axis=1))

    common = {
        "xt_full": xt,
        "ident": ident,
        "wl_comb1": cat(inputs["in_Wl0"], inputs["out_Wl0"]),
        "wr_in1": np.asarray(inputs["in_Wr0"], np.float32),
        "wr_out1": np.asarray(inputs["out_Wr0"], np.float32),
        "bias_pk1": np.concatenate(
            [np.asarray(inputs["in_bl0"], np.float32),
             np.asarray(inputs["out_bl0"], np.float32)])[:, None].copy(),
        "wcx1": np.ascontiguousarray(np.asarray(inputs["comb_W0"], np.float32)[0:128]),
        "wch1": np.ascontiguousarray(np.asarray(inputs["comb_W0"], np.float32)[128:256]),
        "cb1": np.asarray(inputs["comb_b0"], np.float32)[:, None].copy(),
        "wl_comb2": cat(inputs["in_Wl1"], inputs["out_Wl1"]),
        "wr_in2": np.asarray(inputs["in_Wr1"], np.float32),
        "wr_out2": np.asarray(inputs["out_Wr1"], np.float32),
        "bias_pk2": np.concatenate(
            [np.asarray(inputs["in_bl1"], np.float32),
             np.asarray(inputs["out_bl1"], np.float32)])[:, None].copy(),
        "wcx2": np.ascontiguousarray(np.asarray(inputs["comb_W1"], np.float32)[0:128]),
        "wch2": np.ascontiguousarray(np.asarray(inputs["comb_W1"], np.float32)[128:256]),
        "cb2": np.asarray(inputs["comb_b1"], np.float32)[:, None].copy(),
        "fw": np.asarray(inputs["final_W"], np.float32).reshape(128, 1).copy(),
        "fb": np.asarray(inputs["final_b"], np.float32).reshape(1, 1).copy(),
    }
    in_maps = []
    for c in range(NC):
        m = dict(common)
        xl = np.zeros((128, NLOCP), dtype=np.float32)
        xl[:, :NLOC] = x.T[:, c * NLOC:(c + 1) * NLOC]
        m["xt_loc"] = xl
        for d in ("in", "out"):
            dinfo = plan["dirs"][d]
            rc = np.zeros((128, NCH), dtype=np.float32)
            r = dinfo["recip"][c]  # [NLOCP]
            rc[:, :] = r.reshape(NCH, 128).T
            m[f"recip_{d}"] = rc.copy()
            for h in ("A", "B"):
                hinfo = dinfo["halves"][h]
                m[f"stream_{d}_{h}"] = _wrap_idx(hinfo["streams"][c])
                m[f"unperm_{d}_{h}"] = _wrap_idx(hinfo["unperm"][c])
        in_maps.append(m)
    return in_maps


def kernel(**inputs):
    plan = _preprocess(np.asarray(inputs["edge_index_in"]),
                       np.asarray(inputs["edge_index_out"]))
    key = tuple(
        (d, h, tuple(plan["dirs"][d]["halves"][h]["NR"]))
        for d in ("in", "out") for h in ("A", "B"))
    if key not in _CACHE:
        _CACHE[key] = _build_program(plan)
    nc = _CACHE[key]
    in_maps = _make_in_maps(plan, inputs)
    res = bass_utils.run_bass_kernel_spmd(nc, in_maps, core_ids=list(range(NC)))
    out = np.concatenate([r["out"][0] for r in res.results])
    return out.astype(np.float32)
